# revision 8
# baseline (speedup 1.0000x reference)
"""Trainium2 Bass kernel for nn_DebugBertLayer_87093346828840.

Key observation: the reference overwrites q/k/v with the constant 0.01, so
softmax(scores) is uniform and ctx == 0.01 everywhere.  Hence
    attn_out = LN1(hidden + cvec),   cvec = 0.01 * Wo.sum(axis=1) + bo
and the only real device work is the FFN:
    out = LN2( gelu(attn_out @ Wi.T + bi) @ Wf.T + bf + attn_out )

Sharding: pure data-parallel over the 8192 tokens -> 1024 tokens/core on 8
NeuronCores, no collectives.

Matmuls run in fp8-e4m3 DoubleRow perf mode.  Measured on this hardware
(see bench_mm.py): a 512-wide DoubleRow matmul instruction costs ~221 ns
(~107 ns matmult + fully-exposed ldweights), i.e. the per-instruction /
weight-load overhead is comparable to the matmult itself, so the kernel
minimizes matmul instruction count: no hi/lo weight splitting, 512-wide
moving chunks, and one ldweights per 512-wide DoubleRow matmul.
End-to-end rel err vs the fp32 reference: ~1.86e-2 (budget 2e-2,
deterministic inputs).

Scale bookkeeping (all powers of 2, exact):
  a_bf   = LN1(x) * 2^10            (bf16, SBUF; doubles as mm2 residual)
  aT     = a_bf * 2^-6 = LN1*2^4    (fp8; DMA-xbar transpose + ACT cast)
  wi/wf  = W.T * 2^10               (fp8 pair layout)
  psum1  = 2^(4+10) * a@Wi.T        -> gelu(psum * 2^-14), hT fp8 unscaled
  psum2  = 2^10 * h@Wf.T            -> + a_bf (also 2^10) -> LN2 is
                                       scale-invariant, no unscale needed.

Per-core dataflow (token tile = 128 tokens, two token-halves of 512):
  1. LN1 on bf16 x tiles (bn_stats; Newton rsqrt on DVE so ACT only ever
     runs Gelu -> no activation-table reloads); apply -> a_bf (bf16).
  2. DMA-xbar transpose (off-PE) a_bf -> aTb bf16; one wide ACT copy per
     half casts aTb -> aT fp8 (2^-6).
  3. mm1 per half: for each of 24 m-tiles, 3 DoubleRow matmuls (k-pairs,
     512-wide); gelu psum -> fp8 hT.
  4. mm2 per token tile: for each of 12 m-pairs, DoubleRow 512- and
     256-wide chunks with hT stationary -> token-major psum.
  5. psum + a_bf residual, LN2, bf16 out DMA (host upcasts to fp32).
"""

import os
import sys

for _p in ("/opt/trn_rl_repo", "/root/.axon_site/_ro/trn_rl_repo"):
    if os.path.isdir(_p) and _p not in sys.path:
        sys.path.insert(0, _p)

import numpy as np
import ml_dtypes

import concourse.bass as bass
import concourse.bacc as bacc
import concourse.tile as tile
from concourse import mybir
from concourse.bass_utils import run_bass_kernel_spmd

F32 = mybir.dt.float32
BF16 = mybir.dt.bfloat16
FP8 = mybir.dt.float8e4
AF = mybir.ActivationFunctionType
ALU = mybir.AluOpType
DR = mybir.MatmulPerfMode.DoubleRow
BF16NP = ml_dtypes.bfloat16
FP8NP = mybir.dt.np(FP8)

D = 768           # d_model
FF = 3072         # d_ff
NCORE = 8
TOK = 8192        # total tokens (4 x 2048)
TPC = TOK // NCORE  # 1024 tokens per core
KD = D // 128     # 6 k-tiles over d_model
KP = KD // 2      # 3 k-pairs (DoubleRow)
MF = FF // 128    # 24 tiles over d_ff
MP = MF // 2      # 12 m-pairs (DoubleRow)
NT = TPC // 128   # 8 token tiles per core
NTH = NT // 2     # token tiles per half
HALF = TPC // 2   # 512
LN_EPS = 1e-12

SW = 10           # weight scale 2^SW
SA = 4            # fp8 activation scale 2^SA
W_SCALE = float(2.0 ** SW)

_NC_CACHE = {}
LAST_RESULTS = None
RUN_KWARGS = {}


def _ln_tile(nc, pstat, s_t, gb, apply_gb, out=None, out_scale=1.0):
    """LayerNorm over the free dim (768) of s_t [128, 768].

    Writes (s_t - mu) * rstd * out_scale into `out` (defaults to s_t in
    place).  rstd on the Vector engine (bit-trick seed + 2 Newton steps)
    instead of ScalarE Sqrt: the ACT engine then only ever runs Gelu, which
    avoids ~1.3-7.6us activation-table reloads on every Sqrt<->Gelu switch.
    """
    g_b, b_b = gb
    sr = s_t.rearrange("p (n s) -> p n s", s=384)
    stats = pstat.tile([128, 2, 6], F32, tag="stats")
    for i in range(2):
        nc.vector.bn_stats(out=stats[:, i, :], in_=sr[:, i, :])
    mv = pstat.tile([128, 2], F32, tag="mv")
    nc.vector.bn_aggr(out=mv[:], in_=stats[:])
    v = mv[:, 1:2]
    # var + LN_EPS (1e-12) == var in fp32 for any var > ~1e-5, which always
    # holds here (LN inputs are ~N(0,1)); skip the extra DVE pass.
    rst = pstat.tile([128, 1], F32, tag="rst")
    nrt = pstat.tile([128, 1], F32, tag="nrt")
    # y0 = bitcast(0x5f3759df - (bits(v) >> 1)): ~3.4% rsqrt seed
    nc.vector.tensor_scalar(out=rst.bitcast(mybir.dt.int32)[:],
                            in0=v.bitcast(mybir.dt.int32),
                            scalar1=1, scalar2=None,
                            op0=ALU.logical_shift_right)
    nc.vector.tensor_scalar(out=rst.bitcast(mybir.dt.int32)[:],
                            in0=rst.bitcast(mybir.dt.int32)[:],
                            scalar1=-1, scalar2=0x5F3759DF,
                            op0=ALU.mult, op1=ALU.add)
    for _ in range(2):  # y <- y*(1.5 - 0.5*v*y^2); 2 steps -> ~1e-6 rel
        nc.vector.tensor_mul(out=nrt[:], in0=rst[:], in1=rst[:])
        nc.vector.tensor_mul(out=nrt[:], in0=nrt[:], in1=v)
        nc.vector.tensor_scalar(out=nrt[:], in0=nrt[:], scalar1=-0.5,
                                scalar2=1.5, op0=ALU.mult, op1=ALU.add)
        nc.vector.tensor_mul(out=rst[:], in0=rst[:], in1=nrt[:])
    if out_scale != 1.0:
        nc.vector.tensor_scalar(out=rst[:], in0=rst[:], scalar1=out_scale,
                                scalar2=None, op0=ALU.mult)
    dst = s_t if out is None else out
    nc.vector.tensor_scalar(out=dst[:], in0=s_t[:], scalar1=mv[:, 0:1],
                            scalar2=rst[:], op0=ALU.subtract, op1=ALU.mult)
    if apply_gb:
        nc.vector.tensor_mul(out=dst[:], in0=dst[:], in1=g_b[:])
        nc.vector.tensor_add(out=dst[:], in0=dst[:], in1=b_b[:])


def _emit_body(nc, tc, pools, tensors, flags, x_pre=None, emit_wf=None):
    """Emit one full layer computation (one 'rep')."""
    (pw, px, pa_pool, pbig, pstat, pout, ps1, psm) = pools
    (x, y, wi_tiles, wf_tiles, g1_b, b1_b, g2_b, b2_b, bfv_b, bi_sb) = tensors

    a_tiles = [None] * NT
    PHASES = [(0, NTH), (NTH, NT)]
    aT_ph = {}

    def ln1_and_transpose(ph):
        t0p, t1p = PHASES[ph]
        width = (t1p - t0p) * 128
        # aTb/aT[p, kpair, parity, tok]: feature-major LN1 output
        aTb = pbig.tile([128, KP, 2, width], BF16, tag=f"aTb{ph}")
        aT = pbig.tile([128, KP, 2, width], FP8, tag=f"aT{ph}")
        aT_ph[ph] = aT
        for tt, t in enumerate(range(t0p, t1p)):
            if x_pre is not None:
                x_t = x_pre[t]
            else:
                x_t = px.tile([128, D], BF16, tag="xa")
                nc.sync.dma_start(out=x_t[:], in_=x[t * 128:(t + 1) * 128, :])
            a_t = pa_pool.tile([128, D], BF16, tag="ab")
            _ln_tile(nc, pstat, x_t, (g1_b, b1_b), flags["g1b1"],
                     out=a_t, out_scale=W_SCALE)
            a_tiles[t] = a_t
            for k in range(KD):
                # off-PE transpose: DMA crossbar (bf16)
                nc.scalar.dma_start(
                    out=aTb[:, k // 2, k % 2, tt * 128:(tt + 1) * 128],
                    in_=a_t[:, k * 128:(k + 1) * 128], transpose=True)
        # one wide ACT cast per half: aT fp8 = aTb * 2^(SA-SW)
        nc.scalar.activation(out=aT[:], in_=aTb[:], func=AF.Copy,
                             scale=float(2.0 ** (SA - SW)))

    def mm1_phase(ph, hT):
        t0p, t1p = PHASES[ph]
        width = (t1p - t0p) * 128
        off = t0p * 128
        aT = aT_ph[ph]
        for m in range(MF):
            ps_a = ps1.tile([128, 512], F32, tag="hps")
            for kp in range(KP):
                nc.tensor.matmul(
                    ps_a[:, 0:width], wi_tiles[kp][:, :, m * 128:(m + 1) * 128],
                    aT[:, kp, :, :],
                    start=(kp == 0), stop=(kp == KP - 1), perf_mode=DR)
            dst = hT[:, m // 2, m % 2, off:off + width]
            if flags["bi"]:
                nc.scalar.activation(out=dst, in_=ps_a[:, 0:width],
                                     func=AF.Gelu, bias=bi_sb[:, m:m + 1],
                                     scale=float(2.0 ** (-SA - SW)))
            else:
                nc.scalar.activation(out=dst, in_=ps_a[:, 0:width],
                                     func=AF.Gelu,
                                     scale=float(2.0 ** (-SA - SW)))

    # ---- LN1+transpose / mm1, software-pipelined across the two phases ----
    # hT[p, mpair, parity, tok] fp8, unscaled gelu output
    hT = pbig.tile([128, MP, 2, TPC], FP8, tag="hT")
    ln1_and_transpose(0)
    mm1_phase(0, hT)
    if emit_wf is not None:
        emit_wf(0, MP)
    ln1_and_transpose(1)
    mm1_phase(1, hT)

    # ---------------- mm2 + residual + LN2 ----------------
    for t in range(NT):
        ps2 = psm.tile([128, D], F32, tag="yps")
        for mp in range(MP):
            lhsT = hT[:, mp, :, t * 128:(t + 1) * 128]
            # [0:512] fills PSUM bank 0, [512:768] bank 1: separate zero
            # regions, so the two accumulation groups may coexist
            nc.tensor.matmul(ps2[:, 0:512], lhsT,
                             wf_tiles[mp][:, :, 0:512],
                             start=(mp == 0), stop=(mp == MP - 1),
                             perf_mode=DR)
            nc.tensor.matmul(ps2[:, 512:768], lhsT,
                             wf_tiles[mp][:, :, 512:768],
                             start=(mp == 0), stop=(mp == MP - 1),
                             perf_mode=DR)
        s_t = pout.tile([128, D], F32, tag="s")
        nc.vector.tensor_add(out=s_t[:], in0=ps2[:], in1=a_tiles[t][:])
        if flags["bfv"]:
            nc.vector.tensor_add(out=s_t[:], in0=s_t[:], in1=bfv_b[:])
        o_t = pout.tile([128, D], BF16, tag="o")
        _ln_tile(nc, pstat, s_t, (g2_b, b2_b), flags["g2b2"], out=o_t)
        nc.sync.dma_start(out=y[t * 128:(t + 1) * 128, :], in_=o_t[:])


def _bcast_ap(handle, n):
    """AP that broadcasts a [n]-vector across 128 partitions for DMA."""
    return bass.AP(tensor=handle, offset=0, ap=[[0, 128], [1, n]])


def _build(n_reps=1, flag_key=(True, True, True, True)):
    cache_key = (n_reps, flag_key)
    if cache_key in _NC_CACHE:
        return _NC_CACHE[cache_key]
    flags = dict(zip(("g1b1", "g2b2", "bi", "bfv"), flag_key))
    nc = bacc.Bacc("TRN2", target_bir_lowering=False, debug=False,
                   num_devices=NCORE)
    x = nc.dram_tensor("x", [TPC, D], BF16, kind="ExternalInput")
    wi = nc.dram_tensor("wi", [KP, 128, 2, FF], FP8, kind="ExternalInput")
    wf = nc.dram_tensor("wf", [MP, 128, 2, D], FP8, kind="ExternalInput")
    g1 = nc.dram_tensor("g1", [D], F32, kind="ExternalInput")
    b1 = nc.dram_tensor("b1", [D], F32, kind="ExternalInput")
    g2 = nc.dram_tensor("g2", [D], F32, kind="ExternalInput")
    b2 = nc.dram_tensor("b2", [D], F32, kind="ExternalInput")
    bfv = nc.dram_tensor("bfv", [D], F32, kind="ExternalInput")
    bi = nc.dram_tensor("bi", [FF], F32, kind="ExternalInput")
    y = nc.dram_tensor("y", [TPC, D], BF16, kind="ExternalOutput")

    with tile.TileContext(nc) as tc:
        with (
            tc.tile_pool(name="pw", bufs=1) as pw,
            tc.tile_pool(name="px", bufs=NT) as px,
            tc.tile_pool(name="pa", bufs=NT) as pa_pool,
            tc.tile_pool(name="pbig", bufs=1) as pbig,
            tc.tile_pool(name="pstat", bufs=4) as pstat,
            tc.tile_pool(name="pout", bufs=3) as pout,
            tc.tile_pool(name="ps1", bufs=3, space="PSUM") as ps1,
            tc.tile_pool(name="psm", bufs=2, space="PSUM") as psm,
        ):
            # x tiles first: LN1(t0) is the head of the critical chain
            x_pre = []
            for t in range(NTH):
                x_t = px.tile([128, D], BF16, tag="xa")
                nc.sync.dma_start(out=x_t[:], in_=x[t * 128:(t + 1) * 128, :])
                x_pre.append(x_t)

            def bcast(handle, n, tag):
                t = pw.tile([128, n], F32, tag=tag)
                nc.gpsimd.dma_start(out=t[:], in_=_bcast_ap(handle, n))
                return t

            g1_b = bcast(g1, D, "g1") if flags["g1b1"] else None
            b1_b = bcast(b1, D, "b1") if flags["g1b1"] else None
            g2_b = bcast(g2, D, "g2") if flags["g2b2"] else None
            b2_b = bcast(b2, D, "b2") if flags["g2b2"] else None
            bfv_b = bcast(bfv, D, "bfv") if flags["bfv"] else None
            bi_sb = None
            if flags["bi"]:
                bi_sb = pw.tile([128, MF], F32, tag="bi")
                nc.gpsimd.dma_start(
                    out=bi_sb[:],
                    in_=bass.AP(tensor=bi, offset=0, ap=[[1, 128], [128, MF]]))
            scratch = pw.tile([128, 1], F32, tag="scr")
            nc.vector.memset(scratch[:], 0.0)
            # dummy Gelu: hoists the one ACT function-table load into the
            # DMA prologue where it is fully hidden
            nc.scalar.activation(out=scratch[:], in_=scratch[:], func=AF.Gelu,
                                 scale=1.0)

            # DMA order = HBM arrival order: first-half x tiles, then the
            # mm1 weights, then second-half x, then the mm2 weights.
            wi_tiles = []
            for kp in range(KP):
                wt = pw.tile([128, 2, FF], FP8, tag=f"wi{kp}")
                nc.sync.dma_start(out=wt[:], in_=wi[kp])
                wi_tiles.append(wt)
            for t in range(NTH, NT):
                x_t = px.tile([128, D], BF16, tag="xa")
                nc.sync.dma_start(out=x_t[:], in_=x[t * 128:(t + 1) * 128, :])
                x_pre.append(x_t)
            wf_tiles = []
            for mp in range(MP):
                wt = pw.tile([128, 2, D], FP8, tag=f"wf{mp}")
                wf_tiles.append(wt)

            def emit_wf(m0, m1):
                for mp in range(m0, m1):
                    nc.sync.dma_start(out=wf_tiles[mp][:], in_=wf[mp])

            tensors = (x, y, wi_tiles, wf_tiles, g1_b, b1_b, g2_b, b2_b,
                       bfv_b, bi_sb)
            pools = (pw, px, pa_pool, pbig, pstat, pout, ps1, psm)
            if isinstance(n_reps, tuple):  # ("loop", n) -> dynamic Tile loop
                emit_wf(0, MP)
                with tc.For_i(0, n_reps[1], 1):
                    _emit_body(nc, tc, pools, tensors, flags)
            else:
                for i in range(n_reps):
                    _emit_body(nc, tc, pools, tensors, flags,
                               x_pre=x_pre if i == 0 else None,
                               emit_wf=emit_wf if i == 0 else None)

    nc.compile()
    _NC_CACHE[cache_key] = nc
    return nc


def _prep_inputs(hidden_states, Wo, bo, ln1_g, ln1_b, Wi, bi, Wf, bf,
                 ln2_g, ln2_b):
    x = np.asarray(hidden_states, np.float32).reshape(TOK, D)
    Wo = np.asarray(Wo, np.float32)
    Wi = np.asarray(Wi, np.float32)
    Wf = np.asarray(Wf, np.float32)
    cvec = (0.01 * Wo.sum(axis=1) + np.asarray(bo, np.float32))
    x = np.ascontiguousarray((x + cvec[None, :]).astype(BF16NP))
    # wi layout: [kpair, p, parity, f] = Wi.T[(2*kp+i)*128+p, f] * 2^SW
    wi_prep = np.ascontiguousarray(
        (Wi.T * W_SCALE).reshape(KP, 2, 128, FF).transpose(0, 2, 1, 3)
        .astype(FP8NP))
    # wf layout: [mpair, p, parity, j] = Wf.T[(2*mp+i)*128+p, j] * 2^SW
    wf_prep = np.ascontiguousarray(
        (Wf.T * W_SCALE).reshape(MP, 2, 128, D).transpose(0, 2, 1, 3)
        .astype(FP8NP))
    common = {
        "wi": wi_prep, "wf": wf_prep,
        "g1": np.asarray(ln1_g, np.float32),
        "b1": np.asarray(ln1_b, np.float32) * W_SCALE,
        "g2": np.asarray(ln2_g, np.float32),
        "b2": np.asarray(ln2_b, np.float32),
        "bfv": np.asarray(bf, np.float32) * W_SCALE,
        "bi": np.asarray(bi, np.float32),
    }
    in_maps = [dict(common, x=x[c * TPC:(c + 1) * TPC]) for c in range(NCORE)]
    flag_key = (
        not (np.all(ln1_g == 1.0) and np.all(ln1_b == 0.0)),
        not (np.all(ln2_g == 1.0) and np.all(ln2_b == 0.0)),
        bool(np.any(np.asarray(bi) != 0.0)),
        bool(np.any(np.asarray(bf) != 0.0)),
    )
    return in_maps, flag_key


def kernel(hidden_states, Wq, bq, Wk, bk, Wv, bv, Wo, bo, ln1_g, ln1_b,
           Wi, bi, Wf, bf, ln2_g, ln2_b):
    global LAST_RESULTS
    B, S, _ = hidden_states.shape
    in_maps, flag_key = _prep_inputs(hidden_states, Wo, bo, ln1_g, ln1_b,
                                     Wi, bi, Wf, bf, ln2_g, ln2_b)
    nc = _build(RUN_KWARGS.get("n_reps", 1), flag_key)
    res = run_bass_kernel_spmd(nc, in_maps, list(range(NCORE)),
                               **{k: v for k, v in RUN_KWARGS.items()
                                  if k != "n_reps"})
    LAST_RESULTS = res
    out = np.concatenate([np.asarray(res.results[c]["y"], np.float32)
                          for c in range(NCORE)], axis=0)
    return np.ascontiguousarray(out.reshape(B, S, D).astype(np.float32))
